# revision 46
# baseline (speedup 1.0000x reference)
"""Conv2D 3x3 stride-1 pad-1 (NCHW) as implicit GEMM on 8 NeuronCores.

Strategy: data-parallel over batch (32 imgs -> 4 per core). The input is
zero-padded on the host to (*, 128, 58, 58), converted to bf16, and all
4 images DMA into one resident SBUF tile [C=128, 4, 58, 58] (27KB per
partition) so no load ever waits on compute. Weights are preprocessed
host-side to bf16 [I=128, (kh kw o)] so each (tap, ochunk) slice is a
ready [K=128, M=128] stationary operand; bf16 LDWEIGHTS (91ns) hides
fully under the previous 448-col matmul (187ns), unlike fp32r whose
4-byte weight load serializes ~23ns per matmul.

Loop order: image -> row-group (8 rows, free dim 448) -> ochunk -> tap.
Taps innermost means each PSUM group completes every ~1.8us and its
bias-add + output DMA overlap the next group's matmuls -- the drain
stream spreads across the whole kernel instead of piling into a tail.
Output DMAs alternate between the two hardware DGE queues (sync, scalar);
images 2-3 load on the gpsimd software queue, an independent rail.

x (4,128,58,58) bf16 -> out (4,256,56,56) f32 per core; no collectives.
"""

import sys

import numpy as np

if "/opt/trn_rl_repo" not in sys.path:
    sys.path.insert(0, "/opt/trn_rl_repo")

from concourse import bacc, bass, mybir  # noqa: E402
from concourse.bass_utils import run_bass_kernel_spmd  # noqa: E402
from concourse.tile import TileContext, add_dep_helper  # noqa: E402

N_FULL, CIN, H, W = 32, 128, 56, 56
COUT = 256
KH = KW = 3
NCORES = 8
NPER = N_FULL // NCORES  # 4 images per core
HP, WP = H + 2, W + 2  # 58 x 58 padded
ROWS = 8  # output rows per matmul group
NFREE = ROWS * W  # 448 moving free dim
NGROUPS = H // ROWS  # 7
OCH = COUT // 128  # 2 output-channel chunks

_CACHE = {}


def _build_conv():
    f32 = mybir.dt.float32
    bf16 = mybir.dt.bfloat16

    # Bacc (not raw Bass): its compile pipeline legalizes sync waits --
    # TRN2 instructions carry at most one wait slot.
    nc = bacc.Bacc(None, target_bir_lowering=False)

    x_par = nc.declare_dram_parameter("x", [NPER, CIN, HP, WP], bf16, isOutput=False)
    w_par = nc.declare_dram_parameter(
        "wt", [CIN, KH * KW * COUT], bf16, isOutput=False
    )
    # bias comes in host-pretransposed as [128, OCH] so the DMA is a
    # contiguous 8B-per-partition transfer instead of a 256-packet scatter.
    bias_par = nc.declare_dram_parameter("bias", [128, OCH], f32, isOutput=False)
    out_par = nc.declare_dram_parameter("out", [NPER, COUT, H, W], f32, isOutput=True)
    out_flat = out_par.rearrange("n o h w -> n o (h w)")

    with TileContext(nc) as tc:
        with (
            tc.tile_pool(name="const", bufs=1) as cpool,
            tc.tile_pool(name="psum", bufs=7, space="PSUM") as ppool,
            tc.tile_pool(name="outp", bufs=6) as opool,
        ):
            # HAM pre-warm: junk matmuls gated only on a prologue memset run
            # during the initial DMA wait so the PE clock gate is at 8/8
            # (2.4 GHz) when the real stream starts. Results never consumed.
            jnk = cpool.tile([128, 512], f32, tag="jnk")
            # gpsimd clears its framework prologue ~1.5us before vector, so
            # gating the warm-up on a gpsimd memset starts it that much
            # earlier.
            nc.gpsimd.memset(jnk[:], 1.0)
            jnk_mm = jnk.bitcast(bf16)
            ps_jnk = ppool.tile([128, NFREE], f32, tag="ps", name="ps")
            for _ in range(8):
                nc.tensor.matmul(
                    ps_jnk[:],
                    jnk_mm[:, 0:128],
                    jnk_mm[:, 0:NFREE],
                    start=True,
                    stop=True,
                )

            # All four images resident: one [C, n, h, w] tile, 27KB/partition.
            x_sb = cpool.tile([CIN, NPER, HP, WP], bf16, tag="xall", name="xall")
            w_sb = cpool.tile([CIN, KH * KW * COUT], bf16, tag="w", name="w")
            bias_sb = cpool.tile([128, OCH], f32, tag="bias")

            w3_sb = w_sb.rearrange("p (t o) -> p t o", t=KH * KW)
            w3_dr = w_par[:].rearrange("p (t o) -> p t o", t=KH * KW)

            # Three rails, each led by its head-critical tensor: scalar ring
            # leads with w taps 0-4 (one DMA, big packets), the gpsimd
            # software ring leads with w taps 5-8 before the bulk images,
            # and the sync ring leads with image-0 row chunks in consumption
            # order. Images 1-3 are deferred behind the first real matmul so
            # they never contend with the head. Per-DMA completion
            # semaphores release ~1-2.5us after the data and the lag grows
            # with ring depth, so finer JIT chunking of w makes the head
            # slower, not faster.
            nc.sync.dma_start(out=w3_sb[:, 0:5, :], in_=w3_dr[:, 0:5, :])
            nc.scalar.dma_start(out=x_sb[:, 0, 0:18, :], in_=x_par[0, :, 0:18, :])
            nc.sync.dma_start(out=w3_sb[:, 5:9, :], in_=w3_dr[:, 5:9, :])
            nc.sync.dma_start(out=bias_sb[:], in_=bias_par[:])
            nc.scalar.dma_start(out=x_sb[:, 0, 18:26, :], in_=x_par[0, :, 18:26, :])
            nc.scalar.dma_start(out=x_sb[:, 0, 26:42, :], in_=x_par[0, :, 26:42, :])
            nc.scalar.dma_start(out=x_sb[:, 0, 42:58, :], in_=x_par[0, :, 42:58, :])
            img_dmas = [
                nc.gpsimd.dma_start(out=x_sb[:, 1, :, :], in_=x_par[1]),
                nc.gpsimd.dma_start(out=x_sb[:, 2, :, :], in_=x_par[2]),
                nc.gpsimd.dma_start(out=x_sb[:, 3, :, :], in_=x_par[3]),
            ]

            mm_first = None

            # Row groups per image: 7x8 rows, except the last image ends with
            # a 6+2 split so the final accumulate->drain->store chain after
            # the very last matmul is ~4x shorter.
            full_groups = [(g * ROWS, ROWS) for g in range(NGROUPS)]
            tail_groups = full_groups[:-1] + [(48, 6), (54, 2)]

            def emit_mms(ps, n, r0, nrows, oc, taps):
                nonlocal mm_first
                for tap in taps:
                    kh, kw = divmod(tap, KW)
                    mm = nc.tensor.matmul(
                        ps[:],
                        w3_sb[:, tap, oc * 128 : oc * 128 + 128],
                        x_sb[:, n, r0 + kh : r0 + kh + nrows, kw : kw + W],
                        start=(tap == 0),
                        stop=(tap == KH * KW - 1),
                    )
                    if mm_first is None:
                        mm_first = mm

            drain_idx = 0

            # The end-of-kernel semaphore teardown costs ~115ns per DMA per
            # engine, so output stores are merged: two row-groups' bias-adds
            # stage into one [128, 2*NFREE] tile shipped as a single DMA.
            def drain(parts, n, r0, oc, last=False):
                nonlocal drain_idx
                nfree = sum(nrows * W for _, nrows in parts)
                ot = opool.tile([128, nfree], f32, tag="ot", name="ot")
                off = 0
                for ps, nrows in parts:
                    nc.vector.tensor_scalar_add(
                        out=ot[:, off : off + nrows * W],
                        in0=ps[:],
                        scalar1=bias_sb[:, oc : oc + 1],
                    )
                    off += nrows * W
                dst = out_flat[
                    n, oc * 128 : (oc + 1) * 128, r0 * W : r0 * W + nfree
                ]
                if last:
                    # Tail latency: split the final store across both HW
                    # queues.
                    half = nfree // 2
                    nc.sync.dma_start(out=dst[:, 0:half], in_=ot[:, 0:half])
                    nc.scalar.dma_start(out=dst[:, half:nfree], in_=ot[:, half:nfree])
                elif drain_idx % 2 == 0:
                    nc.sync.dma_start(out=dst, in_=ot[:])
                else:
                    nc.scalar.dma_start(out=dst, in_=ot[:])
                drain_idx += 1

            # Head pass: image-0 groups 0-2 run tap-split -- taps 0-4 for
            # all six (group, oc) psums first, taps 5-8 after. This defers
            # the w[5:9] demand ~5.7us and the second x0 chunk ~10us into
            # the stream, past their DMA-completion semaphores (which
            # release 1-2.5us after the data), so the PE never stalls.
            head_units = [(g, oc) for g in (0, 1, 2) for oc in range(OCH)]
            head_ps = {
                u: ppool.tile([128, NFREE], f32, tag="ps", name="ps")
                for u in head_units
            }
            for taps in (range(0, 5), range(5, KH * KW)):
                for g, oc in head_units:
                    emit_mms(head_ps[(g, oc)], 0, g * ROWS, ROWS, oc, taps)
            for oc in range(OCH):
                drain([(head_ps[(0, oc)], ROWS), (head_ps[(1, oc)], ROWS)], 0, 0, oc)
            for oc in range(OCH):
                drain([(head_ps[(2, oc)], ROWS)], 0, 2 * ROWS, oc)

            # Remaining work in merged pairs of row-groups; the final image's
            # 6+2 split groups stay separate so the tail chain is short.
            for n in range(NPER):
                groups = tail_groups if n == NPER - 1 else full_groups
                start_gi = 3 if n == 0 else 0
                pending = list(range(start_gi, len(groups)))
                while pending:
                    gA = pending.pop(0)
                    r0A, nrowsA = groups[gA]
                    merge = (
                        pending
                        and groups[pending[0]][1] == ROWS
                        and nrowsA == ROWS
                    )
                    gB = pending.pop(0) if merge else None
                    pss = {}
                    for oc in range(OCH):
                        for g in ([gA, gB] if merge else [gA]):
                            r0, nrows = groups[g]
                            ps = ppool.tile(
                                [128, nrows * W], f32, tag="ps", name="ps"
                            )
                            emit_mms(ps, n, r0, nrows, oc, range(KH * KW))
                            pss[(g, oc)] = ps
                    for oc in range(OCH):
                        if merge:
                            drain(
                                [(pss[(gA, oc)], ROWS), (pss[(gB, oc)], ROWS)],
                                n,
                                r0A,
                                oc,
                            )
                        else:
                            last = (
                                n == NPER - 1
                                and gA == len(groups) - 1
                                and oc == OCH - 1
                            )
                            drain([(pss[(gA, oc)], nrowsA)], n, r0A, oc, last=last)
            for dma in img_dmas:
                add_dep_helper(
                    dma.ins,
                    mm_first.ins,
                    sync=True,
                    reason="defer bulk image loads past the head",
                )
    nc.compile()
    return nc


def _get_nc():
    if "nc" not in _CACHE:
        _CACHE["nc"] = _build_conv()
    return _CACHE["nc"]


# test-harness hooks: set TRACE=True before calling kernel() to capture an
# NTFF profile; LAST_RESULTS then holds the BassKernelResults.
TRACE = False
LAST_RESULTS = None


def kernel(x, weight, bias):
    global LAST_RESULTS
    import ml_dtypes

    bfl = ml_dtypes.bfloat16
    x = np.ascontiguousarray(np.asarray(x), dtype=np.float32)
    w = np.ascontiguousarray(np.asarray(weight), dtype=np.float32)
    b = np.ascontiguousarray(np.asarray(bias), dtype=np.float32)
    xp = np.pad(x, ((0, 0), (0, 0), (1, 1), (1, 1))).astype(bfl)
    # wt[i, (kh kw o)] = w[o, i, kh, kw]
    wt = np.ascontiguousarray(
        w.transpose(1, 2, 3, 0).reshape(CIN, KH * KW * COUT)
    ).astype(bfl)

    b2 = np.ascontiguousarray(b.reshape(OCH, 128).T)  # [128, OCH]
    per_core = [
        {"x": xp[c * NPER : (c + 1) * NPER], "wt": wt, "bias": b2}
        for c in range(NCORES)
    ]

    kwargs = {}
    if TRACE:
        kwargs = dict(trace=True, trace_cores=[0])
    res = run_bass_kernel_spmd(
        _get_nc(), per_core, core_ids=list(range(NCORES)), **kwargs
    )
    LAST_RESULTS = res
    return np.concatenate([r["out"] for r in res.results], axis=0)


# revision 47
# speedup vs baseline: 1.0148x; 1.0148x over previous
"""Conv2D 3x3 stride-1 pad-1 (NCHW) as implicit GEMM on 8 NeuronCores.

Strategy: data-parallel over batch (32 imgs -> 4 per core). The input is
zero-padded on the host to (*, 128, 58, 58), converted to bf16, and all
4 images DMA into one resident SBUF tile [C=128, 4, 58, 58] (27KB per
partition) so no load ever waits on compute. Weights are preprocessed
host-side to bf16 [I=128, (kh kw o)] so each (tap, ochunk) slice is a
ready [K=128, M=128] stationary operand; bf16 LDWEIGHTS (91ns) hides
fully under the previous 448-col matmul (187ns), unlike fp32r whose
4-byte weight load serializes ~23ns per matmul.

Loop order: image -> row-group (8 rows, free dim 448) -> ochunk -> tap.
Taps innermost means each PSUM group completes every ~1.8us and its
bias-add + output DMA overlap the next group's matmuls -- the drain
stream spreads across the whole kernel instead of piling into a tail.
Output DMAs alternate between the two hardware DGE queues (sync, scalar);
images 2-3 load on the gpsimd software queue, an independent rail.

x (4,128,58,58) bf16 -> out (4,256,56,56) f32 per core; no collectives.
"""

import sys

import numpy as np

if "/opt/trn_rl_repo" not in sys.path:
    sys.path.insert(0, "/opt/trn_rl_repo")

from concourse import bacc, bass, mybir  # noqa: E402
from concourse.bass_utils import run_bass_kernel_spmd  # noqa: E402
from concourse.tile import TileContext, add_dep_helper  # noqa: E402

N_FULL, CIN, H, W = 32, 128, 56, 56
COUT = 256
KH = KW = 3
NCORES = 8
NPER = N_FULL // NCORES  # 4 images per core
HP, WP = H + 2, W + 2  # 58 x 58 padded
ROWS = 8  # output rows per matmul group
NFREE = ROWS * W  # 448 moving free dim
NGROUPS = H // ROWS  # 7
OCH = COUT // 128  # 2 output-channel chunks

_CACHE = {}


def _build_conv():
    f32 = mybir.dt.float32
    bf16 = mybir.dt.bfloat16

    # Bacc (not raw Bass): its compile pipeline legalizes sync waits --
    # TRN2 instructions carry at most one wait slot.
    nc = bacc.Bacc(None, target_bir_lowering=False)

    x_par = nc.declare_dram_parameter("x", [NPER, CIN, HP, WP], bf16, isOutput=False)
    w_par = nc.declare_dram_parameter(
        "wt", [CIN, KH * KW * COUT], bf16, isOutput=False
    )
    # bias comes in host-pretransposed as [128, OCH] so the DMA is a
    # contiguous 8B-per-partition transfer instead of a 256-packet scatter.
    bias_par = nc.declare_dram_parameter("bias", [128, OCH], f32, isOutput=False)
    out_par = nc.declare_dram_parameter("out", [NPER, COUT, H, W], f32, isOutput=True)
    out_flat = out_par.rearrange("n o h w -> n o (h w)")

    with TileContext(nc) as tc:
        with (
            tc.tile_pool(name="const", bufs=1) as cpool,
            tc.tile_pool(name="psum", bufs=7, space="PSUM") as ppool,
            tc.tile_pool(name="outp", bufs=6) as opool,
        ):
            # HAM pre-warm: junk matmuls gated only on a prologue memset run
            # during the initial DMA wait so the PE clock gate is at 8/8
            # (2.4 GHz) when the real stream starts. Results never consumed.
            jnk = cpool.tile([128, 512], f32, tag="jnk")
            # gpsimd clears its framework prologue ~1.5us before vector, so
            # gating the warm-up on a gpsimd memset starts it that much
            # earlier.
            nc.gpsimd.memset(jnk[:], 1.0)
            jnk_mm = jnk.bitcast(bf16)
            ps_jnk = ppool.tile([128, NFREE], f32, tag="ps", name="ps")
            for _ in range(8):
                nc.tensor.matmul(
                    ps_jnk[:],
                    jnk_mm[:, 0:128],
                    jnk_mm[:, 0:NFREE],
                    start=True,
                    stop=True,
                )

            # All four images resident: one [C, n, h, w] tile, 27KB/partition.
            x_sb = cpool.tile([CIN, NPER, HP, WP], bf16, tag="xall", name="xall")
            w_sb = cpool.tile([CIN, KH * KW * COUT], bf16, tag="w", name="w")
            bias_sb = cpool.tile([128, OCH], f32, tag="bias")

            w3_sb = w_sb.rearrange("p (t o) -> p t o", t=KH * KW)
            w3_dr = w_par[:].rearrange("p (t o) -> p t o", t=KH * KW)

            # Three rails, each led by its head-critical tensor: scalar ring
            # leads with w taps 0-4 (one DMA, big packets), the gpsimd
            # software ring leads with w taps 5-8 before the bulk images,
            # and the sync ring leads with image-0 row chunks in consumption
            # order. Images 1-3 are deferred behind the first real matmul so
            # they never contend with the head. Per-DMA completion
            # semaphores release ~1-2.5us after the data and the lag grows
            # with ring depth, so finer JIT chunking of w makes the head
            # slower, not faster.
            nc.sync.dma_start(out=w3_sb[:, 0:5, :], in_=w3_dr[:, 0:5, :])
            nc.scalar.dma_start(out=x_sb[:, 0, 0:18, :], in_=x_par[0, :, 0:18, :])
            nc.sync.dma_start(out=w3_sb[:, 5:9, :], in_=w3_dr[:, 5:9, :])
            nc.sync.dma_start(out=bias_sb[:], in_=bias_par[:])
            nc.scalar.dma_start(out=x_sb[:, 0, 18:26, :], in_=x_par[0, :, 18:26, :])
            nc.scalar.dma_start(out=x_sb[:, 0, 26:42, :], in_=x_par[0, :, 26:42, :])
            nc.scalar.dma_start(out=x_sb[:, 0, 42:58, :], in_=x_par[0, :, 42:58, :])
            img_dmas = [
                nc.gpsimd.dma_start(out=x_sb[:, 1, :, :], in_=x_par[1]),
                nc.gpsimd.dma_start(out=x_sb[:, 2, :, :], in_=x_par[2]),
                nc.gpsimd.dma_start(out=x_sb[:, 3, :, :], in_=x_par[3]),
            ]

            mm_first = None

            # Row groups per image: 7x8 rows, except the last image ends with
            # a 6+2 split so the final accumulate->drain->store chain after
            # the very last matmul is ~4x shorter.
            full_groups = [(g * ROWS, ROWS) for g in range(NGROUPS)]
            tail_groups = full_groups[:-1] + [(48, 6), (54, 2)]

            def emit_mms(ps, n, r0, nrows, oc, taps):
                nonlocal mm_first
                for tap in taps:
                    kh, kw = divmod(tap, KW)
                    mm = nc.tensor.matmul(
                        ps[:],
                        w3_sb[:, tap, oc * 128 : oc * 128 + 128],
                        x_sb[:, n, r0 + kh : r0 + kh + nrows, kw : kw + W],
                        start=(tap == 0),
                        stop=(tap == KH * KW - 1),
                    )
                    if mm_first is None:
                        mm_first = mm

            drain_idx = 0

            # The end-of-kernel semaphore teardown costs ~115ns per DMA per
            # engine, so output stores are merged: two row-groups' bias-adds
            # stage into one [128, 2*NFREE] tile shipped as a single DMA.
            def drain(parts, n, r0, oc, last=False):
                nonlocal drain_idx
                nfree = sum(nrows * W for _, nrows in parts)
                ot = opool.tile([128, nfree], f32, tag="ot", name="ot")
                off = 0
                for ps, nrows in parts:
                    nc.vector.tensor_scalar_add(
                        out=ot[:, off : off + nrows * W],
                        in0=ps[:],
                        scalar1=bias_sb[:, oc : oc + 1],
                    )
                    off += nrows * W
                dst = out_flat[
                    n, oc * 128 : (oc + 1) * 128, r0 * W : r0 * W + nfree
                ]
                if last:
                    # Tail latency: split the final store across both HW
                    # queues.
                    half = nfree // 2
                    nc.sync.dma_start(out=dst[:, 0:half], in_=ot[:, 0:half])
                    nc.scalar.dma_start(out=dst[:, half:nfree], in_=ot[:, half:nfree])
                elif drain_idx % 2 == 0:
                    nc.sync.dma_start(out=dst, in_=ot[:])
                else:
                    nc.scalar.dma_start(out=dst, in_=ot[:])
                drain_idx += 1

            # Head pass: image-0 groups 0-2 run tap-split -- taps 0-4 for
            # all six (group, oc) psums first, taps 5-8 after. This defers
            # the w[5:9] demand ~5.7us and the second x0 chunk ~10us into
            # the stream, past their DMA-completion semaphores (which
            # release 1-2.5us after the data), so the PE never stalls.
            head_units = [(g, oc) for g in (0, 1, 2) for oc in range(OCH)]
            head_ps = {
                u: ppool.tile([128, NFREE], f32, tag="ps", name="ps")
                for u in head_units
            }
            for taps in (range(0, 5), range(5, KH * KW)):
                for g, oc in head_units:
                    emit_mms(head_ps[(g, oc)], 0, g * ROWS, ROWS, oc, taps)
            for g, oc in head_units:
                drain([(head_ps[(g, oc)], ROWS)], 0, g * ROWS, oc)

            for n in range(NPER):
                groups = tail_groups if n == NPER - 1 else full_groups
                for gi, (r0, nrows) in enumerate(groups):
                    if n == 0 and gi < 3:
                        continue
                    for oc in range(OCH):
                        ps = ppool.tile([128, nrows * W], f32, tag="ps", name="ps")
                        emit_mms(ps, n, r0, nrows, oc, range(KH * KW))
                        last = (
                            n == NPER - 1
                            and gi == len(groups) - 1
                            and oc == OCH - 1
                        )
                        drain([(ps, nrows)], n, r0, oc, last=last)
            for dma in img_dmas:
                add_dep_helper(
                    dma.ins,
                    mm_first.ins,
                    sync=True,
                    reason="defer bulk image loads past the head",
                )
    nc.compile()
    return nc


def _get_nc():
    if "nc" not in _CACHE:
        _CACHE["nc"] = _build_conv()
    return _CACHE["nc"]


# test-harness hooks: set TRACE=True before calling kernel() to capture an
# NTFF profile; LAST_RESULTS then holds the BassKernelResults.
TRACE = False
LAST_RESULTS = None


def kernel(x, weight, bias):
    global LAST_RESULTS
    import ml_dtypes

    bfl = ml_dtypes.bfloat16
    x = np.ascontiguousarray(np.asarray(x), dtype=np.float32)
    w = np.ascontiguousarray(np.asarray(weight), dtype=np.float32)
    b = np.ascontiguousarray(np.asarray(bias), dtype=np.float32)
    xp = np.pad(x, ((0, 0), (0, 0), (1, 1), (1, 1))).astype(bfl)
    # wt[i, (kh kw o)] = w[o, i, kh, kw]
    wt = np.ascontiguousarray(
        w.transpose(1, 2, 3, 0).reshape(CIN, KH * KW * COUT)
    ).astype(bfl)

    b2 = np.ascontiguousarray(b.reshape(OCH, 128).T)  # [128, OCH]
    per_core = [
        {"x": xp[c * NPER : (c + 1) * NPER], "wt": wt, "bias": b2}
        for c in range(NCORES)
    ]

    kwargs = {}
    if TRACE:
        kwargs = dict(trace=True, trace_cores=[0])
    res = run_bass_kernel_spmd(
        _get_nc(), per_core, core_ids=list(range(NCORES)), **kwargs
    )
    LAST_RESULTS = res
    return np.concatenate([r["out"] for r in res.results], axis=0)
